# revision 32
# baseline (speedup 1.0000x reference)
"""Trainium2 Bass kernel for NonlinearElectronicEmbedding (segment softmax).

Design ("T2", transposed / padding-free), ~226 us on 8 cores (vs 461 us
for the padded atoms-on-partitions baseline):
  - 512 molecules -> 64 consecutive molecules per core (8 cores). Atoms of
    a core's molecules form one contiguous run (batch_seg sorted); x is
    shipped TRANSPOSED (features on partitions, atoms on the free axis) in
    bf16, so there is no 128-atom padding at all.
  - Prelude computes the k/v tables from E via the ResidualMLPs in
    transposed layout (features on partitions), fusing Wq and kbo@Wq into
    the k-table:  dot(a) = x(a) . (k_mol @ Wq)[seg(a)].
  - Main loop over "supers" of 1024 atoms (s):
      dots  = kqT^T @ xT        (PE, vs all 64 molecules at once, 4 MM of
                                 FD=512 into 2 PSUM banks)
      e     = exp(dots/16)      (one ACT, cross-bank PSUM read, bf16 out)
      S     = e * mask, partial = rowsum(S)   (DVE stt with fused accum)
      anorm += partial; r = 1/(anorm+eps)     (tiny DVE ops)
      then, lagged THREE supers (keeps the PE queue free of just-computed
      deps so it never stalls; molecules < 1024 atoms close by s+1):
      S[s-3] *= r[s-2]; outT[s-3] = v16^T @ S[s-3]  (PE, K=64, 4 MM);
      2 merged ACT copies PSUM->SBUF bf16; 1 DMA out.
  - mask is a host-built fp8e4 0/1 band matrix [64, NCpad] (bs sorted ->
    contiguous band; fp8 halves its traffic, DVE stt is 1x anyway).
    It zeroes the garbage dot rows (wrong molecules) and makes the stt
    accum produce exact per-molecule softmax sums. Softmax shift
    invariance makes the seg_max pass unnecessary (args bounded).
  - x/out DRAM rows are per-super interleaved [p][s][c][j] so each super
    moves with ONE descriptor, 4KB contiguous per partition each way.
  - Host does only layout/dtype work: transpose+bf16 cast in, inverse
    transpose+fp32 cast out; rel err ~4.8e-3 (gate 2e-2).
HBM traffic/core ~ 26+3+26 MB (x + mask + out) -> near memory roofline
(DMA ~83% duty, PE ~82% at its sustained streaming rate).
"""

import numpy as np

F = 256
B = 512
NCORES = 8
BC = B // NCORES  # molecules per core
P = 128
SUP = 1024        # atoms per super-group (2 PSUM banks of dots)
HB = SUP // 2     # 512, one PSUM bank
BETA = 1.702
EPS = 1e-8
INV_SQRT_F = 1.0 / 16.0


def _build_program(nsup):
    import concourse.bacc as bacc
    import concourse.mybir as mybir
    import concourse.tile as tile

    dt = mybir.dt
    f32 = dt.float32
    f16 = dt.bfloat16
    AF = mybir.ActivationFunctionType
    ALU = mybir.AluOpType

    NCpad = nsup * SUP

    nc = bacc.Bacc(trn_type="TRN2")

    f8 = dt.float8e4
    x_h = nc.dram_tensor("x", [P, nsup * 2 * SUP], f16, kind="ExternalInput")
    mk_h = nc.dram_tensor("mk", [BC, NCpad], f8, kind="ExternalInput")
    ev_h = nc.dram_tensor("ev", [1, BC], f32, kind="ExternalInput")
    # all shared weights packed into one blob -> ONE startup DMA
    wb_h = nc.dram_tensor("wb", [P, 3850], f32, kind="ExternalInput")
    out_h = nc.dram_tensor("out", [P, nsup * 2 * SUP], f16,
                           kind="ExternalOutput")

    # per-super interleaved layout: row p holds [s][c][j] so one DMA moves
    # 4KB contiguous per partition per super
    xv = x_h[:].rearrange("p (s c j) -> p s c j", s=nsup, c=2)
    ov = out_h[:].rearrange("p (s c j) -> p s c j", s=nsup, c=2)

    with tile.TileContext(nc) as tc:
        with (
            tc.tile_pool(name="singles", bufs=1) as sg,
            tc.tile_pool(name="xpool", bufs=8) as xp,
            tc.tile_pool(name="mpool", bufs=8) as mp,
            tc.tile_pool(name="epool", bufs=3) as ep,
            tc.tile_pool(name="spool", bufs=6) as sp_,
            tc.tile_pool(name="opool", bufs=6) as op,
            tc.tile_pool(name="rpool", bufs=6) as rp,
        ):
            # weight loads FIRST so the serial MLP prelude starts as early
            # as possible; the early x/mask fetches then fill the DMA idle
            # during the prelude compute (x is not needed until the dots)
            def load(name, h, shape):
                t_ = sg.tile(shape, f32, tag=name, name=name)
                nc.sync.dma_start(out=t_[:], in_=h[:])
                return t_

            ev_sb = load("ev", ev_h, [1, BC])
            wsb = load("wb", wb_h, [P, 3850])

            def wview(off):
                return wsb[:, off:off + 512].rearrange(
                    "p (k m q) -> p k m q", k=2, m=2)

            kw1_sb = wview(0)
            kw2_sb = wview(512)
            vw1_sb = wview(1024)
            vw2_sb = wview(1536)
            woqk_sb = wview(2048)
            wovv_sb = wview(2560)
            bkfs_sb = wsb[:, 3072:3074]
            bkfu_sb = wsb[:, 3074:3076]
            kb1s_sb = wsb[:, 3076:3078]
            kb1u_sb = wsb[:, 3078:3080]
            kb2u_sb = wsb[:, 3080:3082]
            wkf_sb = wsb[0:1, 3082:3338]
            wvf_sb = wsb[0:1, 3338:3594]
            bq_sb = wsb[0:1, 3594:3850]

            early_x = []
            early_m = []
            for s0 in (0, 1):
                if s0 >= nsup:
                    break
                x16e = xp.tile([P, 2, SUP], f16, tag="x16", name="x16e")
                nc.sync.dma_start(out=x16e[:], in_=xv[:, s0, :, :])
                early_x.append(x16e)
                mke = mp.tile([BC, SUP], f8, tag="mk", name="mke")
                nc.sync.dma_start(out=mke[:],
                                  in_=mk_h[:, s0 * SUP:(s0 + 1) * SUP])
                early_m.append(mke)

            ones1 = sg.tile([1, BC], f32)
            nc.vector.memset(ones1[:], 1.0)

            kqT16 = sg.tile([P, 2, BC], f16)   # kqT16[f', c, b]
            v16 = sg.tile([BC, 2, P], f16)     # v16[b, c, f']
            anorm_run = sg.tile([BC, 1], f32)
            nc.vector.memset(anorm_run[:], 0.0)

            # ---- prelude: ResidualMLP in transposed layout ----
            # swish(y) = y * sigmoid(BETA*y); h_psum holds y - b.
            def swishT(c, h_psum, bs_ap, bu_ap, pre, keep_hb=False):
                sig = pre.tile([P, BC], f32, tag=f"sig_{c}", name="sig")
                nc.scalar.activation(sig[:], h_psum[:], AF.Sigmoid,
                                     bias=bs_ap if bs_ap is not None else 0.0,
                                     scale=BETA)
                if bu_ap is not None:
                    hb = pre.tile([P, BC], f32, tag=f"hb_{c}", name="hb")
                    nc.vector.tensor_scalar_add(hb[:], h_psum[:], bu_ap)
                elif keep_hb:
                    hb = pre.tile([P, BC], f32, tag=f"hb_{c}", name="hb")
                    nc.vector.tensor_copy(hb[:], h_psum[:])
                else:
                    hb = h_psum
                s = pre.tile([P, BC], f32, tag=f"s_{c}", name="s")
                nc.vector.tensor_mul(s[:], hb[:], sig[:])
                return (s, hb) if keep_hb else (s, None)

            def resmlp_T(wf_sb, b0s, b0u, w1_sb, b1s, b1u, w2_sb, b2u,
                         pre, ppre, branch):
                h0, s1, h1, s2, h2, s3, hb0 = [], [], [], [], [], [], []
                for c in (0, 1):
                    t_ = ppre.tile([P, BC], f32, tag=f"h0_{c}", name="h0")
                    nc.tensor.matmul(t_[:], wf_sb[0:1, c * P:(c + 1) * P],
                                     ev_sb[:], start=True, stop=True)
                    h0.append(t_)
                for c in (0, 1):
                    s, hb = swishT(
                        f"a{c}", h0[c],
                        b0s[:, c:c + 1] if b0s is not None else None,
                        b0u[:, c:c + 1] if b0u is not None else None,
                        pre, keep_hb=True)
                    s1.append(s)
                    hb0.append(hb if hb is not None else h0[c])
                for m in (0, 1):
                    t_ = ppre.tile([P, BC], f32, tag=f"h1_{m}", name="h1")
                    for k in (0, 1):
                        nc.tensor.matmul(t_[:], w1_sb[:, k, m, :], s1[k][:],
                                         start=(k == 0), stop=(k == 1))
                    h1.append(t_)
                for m in (0, 1):
                    s, _ = swishT(
                        f"b{m}", h1[m],
                        b1s[:, m:m + 1] if b1s is not None else None,
                        b1u[:, m:m + 1] if b1u is not None else None, pre)
                    s2.append(s)
                for m in (0, 1):
                    t_ = ppre.tile([P, BC], f32, tag=f"h2_{m}", name="h2")
                    for k in (0, 1):
                        nc.tensor.matmul(t_[:], w2_sb[:, k, m, :], s2[k][:],
                                         start=(k == 0), stop=(k == 1))
                    h2.append(t_)
                for m in (0, 1):
                    rt = pre.tile([P, BC], f32, tag=f"r_{m}_{branch}", name="rt")
                    nc.vector.tensor_add(rt[:], hb0[m][:], h2[m][:])
                    if b2u is not None:
                        nc.vector.tensor_scalar_add(rt[:], rt[:],
                                                    b2u[:, m:m + 1])
                    sig = pre.tile([P, BC], f32, tag=f"sig3_{m}", name="sig3")
                    nc.scalar.activation(sig[:], rt[:], AF.Sigmoid, bias=0.0,
                                         scale=BETA)
                    s = pre.tile([P, BC], f32, tag=f"s3_{m}_{branch}", name="s3")
                    nc.vector.tensor_mul(s[:], rt[:], sig[:])
                    s3.append(s)
                return s3

            with (
                tc.tile_pool(name="pre", bufs=2) as pre,
                tc.tile_pool(name="ppre", bufs=1, space="PSUM") as ppre,
                tc.tile_pool(name="ptab", bufs=1, space="PSUM") as ptab,
            ):
                s3k = resmlp_T(wkf_sb, bkfs_sb, bkfu_sb, kw1_sb, kb1s_sb,
                               kb1u_sb, kw2_sb, kb2u_sb, pre, ppre, "k")
                s3v = resmlp_T(wvf_sb, None, None, vw1_sb, None, None,
                               vw2_sb, None, pre, ppre, "v")
                # kqT[g, b] = sum_h s3k[h, b] * woq[h, g] + bq[g]
                pkq = ptab.tile([P, 2, BC], f32, tag="pkq")
                for c in (0, 1):
                    nc.tensor.matmul(pkq[:, c, :], woqk_sb[:, 0, c, :],
                                     s3k[0][:], start=True, stop=False)
                    nc.tensor.matmul(pkq[:, c, :], woqk_sb[:, 1, c, :],
                                     s3k[1][:], start=False, stop=False)
                    nc.tensor.matmul(pkq[:, c, :],
                                     bq_sb[0:1, c * P:(c + 1) * P],
                                     ones1[:], start=False, stop=True)
                nc.vector.tensor_copy(kqT16[:], pkq[:])
                # v16[b, f'] (chunked) = sum_h s3v[h, b] * wov[h, f']
                pv = ptab.tile([BC, 2, P], f32, tag="pv")
                for c in (0, 1):
                    for k in (0, 1):
                        nc.tensor.matmul(pv[:, c, :], s3v[k][:],
                                         wovv_sb[:, k, c, :],
                                         start=(k == 0), stop=(k == 1))
                nc.vector.tensor_copy(v16[:], pv[:])

            with (
                tc.tile_pool(name="pdot", bufs=2, space="PSUM") as pd_pool,
                tc.tile_pool(name="pout", bufs=2, space="PSUM") as po_pool,
            ):
                S_tiles = [None] * nsup
                r_tiles = [None] * nsup

                def pass2(s):
                    # S[s] *= r (all molecules of super s closed by now)
                    St = S_tiles[s]
                    rt = r_tiles[min(s + 1, nsup - 1)]
                    nc.vector.tensor_scalar_mul(St[:], St[:], rt[:])
                    out16 = op.tile([P, 2, SUP], f16, tag="out16")
                    for c in (0, 1):
                        po = po_pool.tile([P, 2, HB], f32, tag="po")
                        for b in (0, 1):
                            nc.tensor.matmul(
                                po[:, b, :], v16[:, c, :],
                                St[:, b * HB:(b + 1) * HB],
                                start=True, stop=True)
                        dst = out16[:, c, :].rearrange("p (b j) -> p b j", b=2)
                        nc.scalar.activation(dst, po[:], AF.Copy)
                    o_tiles[s] = out16

                x_tiles = [None] * nsup
                m_tiles = [None] * nsup
                o_tiles = [None] * nsup

                def dma_out(j):
                    # deferred one iteration so its copies are already done
                    # and the in-order SP queue never stalls on this trigger
                    nc.sync.dma_start(out=ov[:, j, :, :], in_=o_tiles[j][:])

                def fetch(s):
                    x16 = xp.tile([P, 2, SUP], f16, tag="x16")
                    nc.sync.dma_start(out=x16[:], in_=xv[:, s, :, :])
                    x_tiles[s] = x16
                    mk = mp.tile([BC, SUP], f8, tag="mk")
                    nc.sync.dma_start(out=mk[:],
                                      in_=mk_h[:, s * SUP:(s + 1) * SUP])
                    m_tiles[s] = mk

                for s0 in range(min(2, nsup)):
                    x_tiles[s0], m_tiles[s0] = early_x[s0], early_m[s0]
                for s in range(nsup):
                    if s >= 5:
                        dma_out(s - 5)
                    if s + 2 < nsup:
                        fetch(s + 2)
                    if s >= 4:
                        pass2(s - 4)
                    x16, mk = x_tiles[s], m_tiles[s]

                    pd = pd_pool.tile([BC, 2, HB], f32, tag="pd")
                    for c in (0, 1):
                        for b in (0, 1):
                            nc.tensor.matmul(
                                pd[:, b, :], kqT16[:, c, :],
                                x16[:, c, b * HB:(b + 1) * HB],
                                start=(c == 0), stop=(c == 1),
                                skip_group_check=True)
                    e16 = ep.tile([BC, SUP], f16, tag="e16")
                    nc.scalar.activation(
                        e16[:].rearrange("p (b j) -> p b j", b=2), pd[:],
                        AF.Exp, bias=0.0, scale=INV_SQRT_F)
                    St = sp_.tile([BC, SUP], f16, tag="St")
                    part = rp.tile([BC, 1], f32, tag="part")
                    nc.vector.scalar_tensor_tensor(
                        St[:], e16[:], 1.0, mk[:], ALU.mult, ALU.mult,
                        accum_out=part[:])
                    S_tiles[s] = St
                    nc.vector.tensor_add(anorm_run[:], anorm_run[:], part[:])
                    rt = rp.tile([BC, 1], f32, tag="rt")
                    nc.vector.tensor_scalar_add(rt[:], anorm_run[:], EPS)
                    nc.vector.reciprocal(rt[:], rt[:])
                    r_tiles[s] = rt
                if nsup >= 5:
                    dma_out(nsup - 5)
                for j in range(max(0, nsup - 4), nsup):
                    pass2(j)
                    dma_out(j)

    nc.compile()
    return nc


def _prep_host(x, E, batch_seg, Wq, Wkf, bkf, Wvf, kW1, kb1, kW2, kb2, kWo,
               kbo, vW1, vW2, vWo):
    f32 = np.float32
    import ml_dtypes
    f16 = ml_dtypes.bfloat16
    bs = np.asarray(batch_seg).astype(np.int64)
    x = np.asarray(x, dtype=f32)
    N = x.shape[0]
    core_bounds = np.searchsorted(bs, np.arange(NCORES + 1) * BC, side="left")
    NCmax = int(np.max(np.diff(core_bounds)))
    nsup = max(1, -(-NCmax // SUP))
    NCpad = nsup * SUP

    xts, mks, evs = [], [], []
    E32 = np.asarray(E, dtype=f32)
    for c in range(NCORES):
        n0, n1 = core_bounds[c], core_bounds[c + 1]
        nc_ = n1 - n0
        xt = np.zeros((2 * P, NCpad), dtype=f16)
        xt[:, :nc_] = x[n0:n1].T.astype(f16)
        # interleave: [c_chunk*128+p, s*SUP+j] -> [p, s*(2*SUP)+c_chunk*SUP+j]
        xt = np.ascontiguousarray(
            xt.reshape(2, P, nsup, SUP).transpose(1, 2, 0, 3).reshape(P, -1))
        f8 = ml_dtypes.float8_e4m3fn
        mk = np.zeros((BC, NCpad), dtype=f8)
        mk[:, :nc_] = (bs[n0:n1][None, :]
                       == (np.arange(BC) + c * BC)[:, None]).astype(f8)
        xts.append(xt)
        mks.append(mk)
        evs.append(np.ascontiguousarray(E32[c * BC:(c + 1) * BC].reshape(1, BC)))

    def pack_w(W):
        A = np.asarray(W, dtype=f32)
        return np.ascontiguousarray(A.reshape(2, P, 2, P).transpose(3, 2, 0, 1))

    def pack_hw(M):
        # M [F(h), F(g)] -> [P(h'), k(h-half), c(g-half), P(g')]
        return np.ascontiguousarray(
            M.reshape(2, P, 2, P).transpose(1, 0, 2, 3))

    def pack_b(v, scale):
        a = (np.asarray(v, dtype=f32) * f32(scale)).astype(f32)
        return np.ascontiguousarray(a.reshape(2, P).T)

    Wq_, kWo_, vWo_ = (np.asarray(a, dtype=f32) for a in (Wq, kWo, vWo))
    woq = (kWo_.T @ Wq_).astype(f32)   # [h, g]
    wov = vWo_.T.astype(f32)           # [h, f]
    wb = np.zeros((P, 3850), dtype=f32)
    wb[:, 0:512] = pack_w(kW1).reshape(P, 512)
    wb[:, 512:1024] = pack_w(kW2).reshape(P, 512)
    wb[:, 1024:1536] = pack_w(vW1).reshape(P, 512)
    wb[:, 1536:2048] = pack_w(vW2).reshape(P, 512)
    wb[:, 2048:2560] = pack_hw(woq).reshape(P, 512)
    wb[:, 2560:3072] = pack_hw(wov).reshape(P, 512)
    wb[:, 3072:3074] = pack_b(bkf, BETA)
    wb[:, 3074:3076] = pack_b(bkf, 1.0)
    wb[:, 3076:3078] = pack_b(kb1, BETA)
    wb[:, 3078:3080] = pack_b(kb1, 1.0)
    wb[:, 3080:3082] = pack_b(kb2, 1.0)
    wb[0, 3082:3338] = np.asarray(Wkf, dtype=f32).reshape(F)
    wb[0, 3338:3594] = np.asarray(Wvf, dtype=f32).reshape(F)
    wb[0, 3594:3850] = (np.asarray(kbo, dtype=f32) @ Wq_).reshape(F)
    weights = dict(wb=np.ascontiguousarray(wb))
    return nsup, xts, mks, evs, weights, core_bounds


_CACHE = {}
LAST_RESULT = None


def kernel(x, E, num_batch, batch_seg, Wq, Wkf, bkf, Wvf, kW1, kb1, kW2, kb2,
           kWo, kbo, vW1, vW2, vWo, **_ignored):
    from concourse.bass_utils import run_bass_kernel_spmd

    nsup, xts, mks, evs, weights, core_bounds = _prep_host(
        x, E, batch_seg, Wq, Wkf, bkf, Wvf, kW1, kb1, kW2, kb2, kWo, kbo,
        vW1, vW2, vWo)

    if nsup not in _CACHE:
        _CACHE[nsup] = _build_program(nsup)
    nc = _CACHE[nsup]

    in_maps = [
        dict(weights, x=xts[c], mk=mks[c], ev=evs[c])
        for c in range(NCORES)
    ]
    res = run_bass_kernel_spmd(nc, in_maps, core_ids=list(range(NCORES)))
    global LAST_RESULT
    LAST_RESULT = res

    NCpad = nsup * SUP
    N = np.asarray(x).shape[0]
    out = np.empty((N, F), dtype=np.float32)
    for c in range(NCORES):
        n0, n1 = core_bounds[c], core_bounds[c + 1]
        o = np.asarray(res.results[c]["out"])
        # [p, s*(2*SUP)+cc*SUP+j] -> [cc*128+p, s*SUP+j]
        oT = o.reshape(P, nsup, 2, SUP).transpose(2, 0, 1, 3).reshape(F, NCpad)
        out[n0:n1] = oT[:, :n1 - n0].T.astype(np.float32)
    return out

